# revision 20
# baseline (speedup 1.0000x reference)
"""BitLinear (BitNet a4.8-style) Trainium2 kernel.

Computes  out = act_quant_int4(x) @ ste_ternary(w).T  for
x:[8192,4096] f32, w:[4096,4096] f32, on 8 NeuronCores.

Math structure exploited:
  - act_quant_int4(x) rows are  k/s_t  with integer k in [-7,7],
    s_t = 7/amax_t  (per-token).  The clip to [-8,7] is a no-op since
    |x*s| <= 7 by construction.
  - ste_ternary(w) = q * scale with q in {-1,0,1},
    scale = max(mean|w|, 1e-8)  (global scalar).
  - So out[t,o] = (scale * amax_t / 7) * sum_i k[t,i] * q[o,i].
    The inner sum is an exact small-integer dot product: we run it on the
    PE array in fp8 (e4m3 holds -8..8 and -1..1 exactly; DoubleRow fp8
    accumulates exactly in fp32 PSUM), then scale rows by
    f_t = scale*amax_t/7 during PSUM eviction (bf16 out, cast to f32 on
    host; 2^-9 relative rounding vs the 2e-2 gate).

Three launches on 8 cores:
  1. scale pass: per-core partial |w| sums over a 1/8 row shard of wT,
     reduced in 128-element chunks; host finishes the reduction in f64
     and forms the exact global ternary scale.
  2. w-quant pass: each core ternarizes a 1/8 row shard of wT into fp8
     {-1,0,+1}, passes split across DVE/GpSimd with the cast+store on
     the ACT engine/queue; host gathers the full quantized wT (16.7 MB).
  3. main pass, data-parallel over tokens x8: each core takes
     x[1024,4096] f32 + the full pre-quantized wT[4096,4096] fp8, int4-
     quantizes x on the fly (abs-max reduce -> scale -> round via the
     +-1.5*2^23 magic trick), xbar-transposes the fp8 activations in
     pairs (ACT queue), and runs DoubleRow fp8 matmuls, scaling rows
     into bf16 during PSUM eviction.  The sweep is oc-major so each
     PSUM bank is reused ~1.7us after its eviction starts; x loads and
     wq streaming share the SP queue (x0, x1, then wq so the PE can
     start early); output stores go through the GpSimd SWDGE queue so
     their semaphore waits never head-block a load.

w is transposed on the host once (input marshalling) so the contraction
dim lands on SBUF partitions for both operands.
"""

import numpy as np
from contextlib import ExitStack

import concourse.bacc as bacc
import concourse.bass as bass
import concourse.mybir as mybir
import concourse.tile as tile
from concourse.bass_utils import run_bass_kernel_spmd

F32 = mybir.dt.float32
BF16 = mybir.dt.bfloat16
FP8 = mybir.dt.float8e4
ALU = mybir.AluOpType
ACTF = mybir.ActivationFunctionType

TOK, DIN, DOUT = 8192, 4096, 4096
NCORES = 8
TG, OG = 8, 1            # token shards x out-feature shards (data parallel)
TSH = TOK // TG          # 1024 tokens per core
OSH = DOUT // OG         # 4096 out features per core
NT = TSH // 128          # 8 token tiles per core
NKQ = 8                  # w held in 8 chunks of 2 s-planes (pipelining)
WSEG = DIN // NCORES     # 512 wT rows per core in launches 1/2
MAGIC = 12582912.0       # 1.5*2^23: float add/sub round-to-nearest-int trick
CLAMP = float(np.nextafter(np.float32(1.5), np.float32(0.0)))
EPS = 1e-8

_CACHE = {}


def _build_scale_nc():
    """Launch 1: per-core partial |w| sums, in 128-element chunks so the
    fp32 accumulation error stays ~1e-7 relative (host finishes in f64)."""
    nc = bacc.Bacc("TRN2", target_bir_lowering=False, debug=False,
                   num_devices=NCORES)
    wseg = nc.dram_tensor("wseg", [WSEG, DIN], F32,
                          kind="ExternalInput").ap()
    psums = nc.dram_tensor("psums", [128, 128], F32,
                           kind="ExternalOutput").ap()
    with tile.TileContext(nc) as tc, ExitStack() as ctx:
        pool = ctx.enter_context(tc.tile_pool(name="w", bufs=4))
        spool = ctx.enter_context(tc.tile_pool(name="s", bufs=1))
        sums = spool.tile([128, 8, 16], F32)
        for i in range(8):
            r0, c0 = (i // 2) * 128, (i % 2) * 2048
            wt = pool.tile([128, 16, 128], F32)
            nc.sync.dma_start(
                out=wt,
                in_=wseg[r0:r0 + 128, c0:c0 + 2048].rearrange(
                    "p (a b) -> p a b", a=16))
            nc.vector.tensor_reduce(
                out=sums[:, i, :], in_=wt, axis=mybir.AxisListType.X,
                op=ALU.add, apply_absolute_value=True)
        nc.sync.dma_start(out=psums, in_=sums.rearrange("p a b -> p (a b)"))
    nc.compile()
    return nc


def _build_wquant_nc():
    """Launch 2: ternarize a [512, 4096] row shard of wT into fp8.
    round(clip(y,-1,1)) == round(clamp(y, +-CLAMP)) for |y|<=2.1."""
    nc = bacc.Bacc("TRN2", target_bir_lowering=False, debug=False,
                   num_devices=NCORES)
    wseg = nc.dram_tensor("wseg", [WSEG, DIN], F32,
                          kind="ExternalInput").ap()
    sca = nc.dram_tensor("sca", [128, 2], F32, kind="ExternalInput").ap()
    wq8 = nc.dram_tensor("wq8", [WSEG, DIN], FP8,
                         kind="ExternalOutput").ap()
    with tile.TileContext(nc) as tc, ExitStack() as ctx:
        const = ctx.enter_context(tc.tile_pool(name="const", bufs=1))
        pool = ctx.enter_context(tc.tile_pool(name="w", bufs=8))
        qpool = ctx.enter_context(tc.tile_pool(name="q", bufs=4))
        scat = const.tile([128, 2], F32)
        nc.sync.dma_start(out=scat, in_=sca)
        # 8 chunks; both tensor_scalar passes stay on DVE (it runs them
        # in 2x mode, ~1.1us per pass -- 16 passes fit under the 23.3us
        # load stream and same-engine chaining avoids cross-engine
        # semaphore latency); cast + store ride the ACT engine/queue.
        for i in range(8):
            r0, c0 = (i // 2) * 128, (i % 2) * 2048
            wt = pool.tile([128, 2048], F32, tag="wt")
            nc.sync.dma_start(
                out=wt, in_=wseg[r0:r0 + 128, c0:c0 + 2048])
            nc.vector.tensor_scalar(
                out=wt, in0=wt, scalar1=scat[:, 0:1], scalar2=CLAMP,
                op0=ALU.mult, op1=ALU.min)
            nc.vector.tensor_scalar(
                out=wt, in0=wt, scalar1=-CLAMP, scalar2=MAGIC,
                op0=ALU.max, op1=ALU.add)
            qt = qpool.tile([128, 2048], FP8, tag="qt")
            nc.scalar.activation(out=qt, in_=wt, func=ACTF.Copy,
                                 bias=-MAGIC, scale=1.0)
            nc.scalar.dma_start(out=wq8[r0:r0 + 128, c0:c0 + 2048], in_=qt)
    nc.compile()
    return nc


def _build_main_nc():
    nc = bacc.Bacc("TRN2", target_bir_lowering=False, debug=False,
                   num_devices=NCORES)
    xs = nc.dram_tensor("xs", [TSH, DIN], F32, kind="ExternalInput").ap()
    # Pre-quantized w in pair-interleaved layout: wts8[p, s, b, o] is
    # q_{o,i} for i = s*256 + 2p + b.  This matches what the fp8-pair
    # (uint16) xbar DMA transpose produces for the activations, so the
    # contraction index mapping agrees between lhsT and rhs.
    wts8 = nc.dram_tensor("wts8", [128, 16, 2, OSH], FP8,
                          kind="ExternalInput").ap()
    sca = nc.dram_tensor("sca", [128, 2], F32, kind="ExternalInput").ap()
    out = nc.dram_tensor("out", [TSH, OSH], BF16, kind="ExternalOutput").ap()

    with tile.TileContext(nc) as tc, ExitStack() as ctx:
        const = ctx.enter_context(tc.tile_pool(name="const", bufs=1))
        wqpool = ctx.enter_context(tc.tile_pool(name="wqp", bufs=NKQ))
        xpool = ctx.enter_context(tc.tile_pool(name="xp", bufs=3))
        k8pool = ctx.enter_context(tc.tile_pool(name="k8p", bufs=2))
        ktpool = ctx.enter_context(tc.tile_pool(name="ktp", bufs=4))
        smalls = ctx.enter_context(tc.tile_pool(name="smalls", bufs=4))
        opool = ctx.enter_context(tc.tile_pool(name="osb", bufs=3))
        psum_m = ctx.enter_context(
            tc.tile_pool(name="psm", bufs=8, space="PSUM"))

        scat = const.tile([128, 2], F32)
        nc.sync.dma_start(out=scat, in_=sca)
        w_scale = scat[:, 1:2]

        # Anti-diagonal permutation for reversing per-partition vectors
        # (SwInterleave reverses stationary columns; the host feeds token
        # rows pre-reversed so PSUM comes out ascending, and f crosses the
        # reversal via a tiny R @ f matmul).
        rmat = const.tile([128, 128], F32)
        nc.gpsimd.memset(rmat, 0.0)
        nc.gpsimd.affine_select(
            out=rmat, in_=rmat, compare_op=ALU.not_equal, fill=1.0,
            base=-127, pattern=[[1, 128]], channel_multiplier=1)

        xts = [None] * NT

        def load_x(t):
            xt = xpool.tile([128, DIN], F32, tag="xt", name=f"xt{t}")
            for h in range(2):
                nc.sync.dma_start(
                    out=xt[:, h * 2048:(h + 1) * 2048],
                    in_=xs[t * 128:(t + 1) * 128, h * 2048:(h + 1) * 2048])
            xts[t] = xt

        # SP queue order: x0, x1, x2, wq0..7, then one x per iteration.
        # x-tiles must land ~11us before their sweep starts (the
        # amax->quant->cast->transpose prep chain), and the wq chunks
        # gate tile 0's s-major sweep, which holds all 8 PSUM banks
        # until its last chunk arrives; x3..x7 stream in behind wq well
        # ahead of their sweeps.
        load_x(0)
        load_x(1)
        load_x(2)
        wq = []
        for q in range(NKQ):
            wqt = wqpool.tile([128, 2, 2, OSH], FP8, tag="wq",
                              name=f"wq{q}")
            nc.sync.dma_start(out=wqt, in_=wts8[:, 2 * q:2 * q + 2, :, :])
            wq.append(wqt)

        # Software pipeline: iteration t preps tile t (quant/transpose)
        # and runs the matmul sweep + evictions for tile t-1.
        kts_all = [None] * NT
        fap_all = [None] * NT

        def prep(tt):
            xt = xts[tt]
            amax2 = smalls.tile([128, 2], F32, tag="amax2")
            for h in range(2):
                nc.vector.tensor_reduce(
                    out=amax2[:, h:h + 1],
                    in_=xt[:, h * 2048:(h + 1) * 2048],
                    axis=mybir.AxisListType.X, op=ALU.max,
                    apply_absolute_value=True)
            amax = smalls.tile([128, 1], F32, tag="amax")
            nc.vector.tensor_reduce(
                out=amax, in_=amax2, axis=mybir.AxisListType.X, op=ALU.max)
            nc.vector.tensor_scalar_max(amax, amax, EPS)
            s_ap = smalls.tile([128, 1], F32, tag="s_ap")
            nc.vector.reciprocal(out=s_ap, in_=amax)        # 1/amax
            nc.vector.tensor_scalar_mul(s_ap, s_ap, 7.0)    # s = 7/amax
            f_ap = smalls.tile([128, 1], F32, tag="f_ap")
            nc.vector.tensor_scalar(
                out=f_ap, in0=amax, scalar1=1.0 / 7.0, scalar2=w_scale,
                op0=ALU.mult, op1=ALU.mult)                 # scale*amax/7
            fap_all[tt] = f_ap
            # y = x*s + MAGIC (in-place; integer part is k+MAGIC) on the
            # otherwise-idle GpSimd; ACT subtracts MAGIC and casts to fp8;
            # the ACT-queue xbar DMA then block-transposes fp8 PAIRS (as
            # uint16): kt[p, s, t] = (k[t, s*256+2p], k[t, s*256+2p+1]).
            k8 = k8pool.tile([128, DIN], FP8, tag="k8")
            kts = [ktpool.tile([128, 8, 128], BF16, tag="kt",
                               name=f"kt{tt}_{h}") for h in range(2)]
            for h in range(2):
                for ib in range(4):
                    c0 = h * 2048 + ib * 512
                    nc.vector.tensor_scalar(
                        out=xt[:, c0:c0 + 512], in0=xt[:, c0:c0 + 512],
                        scalar1=s_ap, scalar2=MAGIC,
                        op0=ALU.mult, op1=ALU.add)
                nc.scalar.activation(
                    out=k8[:, h * 2048:(h + 1) * 2048],
                    in_=xt[:, h * 2048:(h + 1) * 2048],
                    func=ACTF.Copy, bias=-MAGIC, scale=1.0)
                nc.scalar.dma_start(
                    out=kts[h],
                    in_=k8.bitcast(BF16)[:, h * 1024:(h + 1) * 1024],
                    transpose=True)
            kts_all[tt] = kts

        def sweep(tt):
            """DoubleRow fp8 matmuls for tile tt + per-oc evictions.

            The f-reversal matmul runs here (not in prep) so it never
            head-blocks the PE queue on a later tile's amax chain.
            Tile 0 runs s-major (each wq chunk consumed as it lands
            during the fill); later tiles run oc-major so each PSUM bank
            is freed ~12us before its next reuse.
            """
            kts = kts_all[tt]
            pss = [psum_m.tile([128, 512], F32, tag="psm",
                               name=f"ps{tt}_{i}")
                   for i in range(8)]
            f_rev = smalls.tile([128, 1], F32, tag="f_rev")

            def frev_mm():
                # f follows the (reversed) fed row order; PSUM rows come
                # out in token order, so reverse f with the permutation
                # matmul.  It borrows the first column of bank 7, which
                # is not accumulated into until the oc7 block ~11us
                # later, so it costs no PSUM slot and no PE stall.  The
                # copy-out rides GpSimd: like the evictions it is sweep-
                # side work, and GpSimd runs nothing x-gated, so it can
                # never head-block behind a late x tile.
                fp = pss[7][:, 0:1]
                nc.tensor.matmul(fp, rmat, fap_all[tt], start=True,
                                 stop=True)
                nc.vector.tensor_copy(out=f_rev, in_=fp)

            def mm(s, oc):
                lhsT = kts[s // 8][:, s % 8, :].bitcast(FP8).rearrange(
                    "p (i m) -> p i m", i=2)
                nc.tensor.matmul(
                    pss[oc], lhsT,
                    wq[s // 2][:, s % 2, :, oc * 512:(oc + 1) * 512],
                    start=(s == 0), stop=(s == 15),
                    perf_mode=mybir.MatmulPerfMode.DoubleRowSwInterleave)

            def evict(oc, osb):
                # GpSimd cannot read PSUM, so evictions split ACT/DVE.
                # They run in pass b, ~7us after the x-gated prep chain
                # on these engines has drained, so head-blocking is
                # bounded to catch-up transients.
                j = oc % 4
                if oc % 2 == 0:
                    nc.scalar.activation(
                        out=osb[:, j * 512:(j + 1) * 512], in_=pss[oc],
                        func=ACTF.Copy, bias=0.0, scale=f_rev)
                else:
                    nc.vector.tensor_scalar(
                        out=osb[:, j * 512:(j + 1) * 512], in0=pss[oc],
                        scalar1=f_rev, scalar2=None, op0=ALU.mult)

            def store(half, osb):
                nc.gpsimd.dma_start(
                    out=out[tt * 128:(tt + 1) * 128,
                            half * 2048:(half + 1) * 2048],
                    in_=osb)

            if tt == 0:
                frev_mm()
                for s in range(16):
                    for oc in range(8):
                        mm(s, oc)
                for half in range(2):
                    osb = opool.tile([128, 2048], BF16, tag="osb")
                    for oc in range(half * 4, half * 4 + 4):
                        evict(oc, osb)
                    store(half, osb)
            else:
                # Two half-K passes: pass a (s 0..7) needs only kt[h0],
                # giving the h1 quant/cast/transpose chain ~7us of slack;
                # pass b completes the accumulation and evicts per-oc.
                osb = None
                for oc in range(8):
                    for s in range(8):
                        mm(s, oc)
                    if oc == 0:
                        frev_mm()
                for oc in range(8):
                    if oc % 4 == 0:
                        osb = opool.tile([128, 2048], BF16, tag="osb")
                    for s in range(8, 16):
                        mm(s, oc)
                    evict(oc, osb)
                    if oc % 4 == 3:
                        store(oc // 4, osb)

        prep(0)
        prep(1)
        sweep(0)
        for tt in range(2, NT):
            if tt + 1 < NT:
                load_x(tt + 1)
            prep(tt)
            sweep(tt - 1)
        sweep(NT - 1)
    nc.compile()
    return nc


def _get_ncs():
    if "scale" not in _CACHE:
        _CACHE["scale"] = _build_scale_nc()
    if "wquant" not in _CACHE:
        _CACHE["wquant"] = _build_wquant_nc()
    if "main" not in _CACHE:
        _CACHE["main"] = _build_main_nc()
    return _CACHE["scale"], _CACHE["wquant"], _CACHE["main"]


def kernel(x: np.ndarray, latent_weight: np.ndarray,
           _collect=None) -> np.ndarray:
    x = np.ascontiguousarray(x, dtype=np.float32)
    wT = np.ascontiguousarray(latent_weight.T.astype(np.float32))
    nc_scale, nc_wq, nc_main = _get_ncs()
    core_ids = list(range(NCORES))
    fp8np = mybir.dt.np(FP8)

    segs = [np.ascontiguousarray(wT[c * WSEG:(c + 1) * WSEG, :])
            for c in core_ids]
    in1 = [{"wseg": segs[c]} for c in core_ids]
    r1 = run_bass_kernel_spmd(nc_scale, in1, core_ids=core_ids)
    total = np.float64(0.0)
    for c in core_ids:
        total += r1.results[c]["psums"].astype(np.float64).sum()
    mean = np.float32(total / (DIN * DOUT))
    scale = np.maximum(mean, np.float32(EPS))
    inv_scale = np.float32(1.0) / scale

    sca = np.empty((128, 2), dtype=np.float32)
    sca[:, 0] = inv_scale
    sca[:, 1] = scale
    in2 = [{"wseg": segs[c], "sca": sca} for c in core_ids]
    r2 = run_bass_kernel_spmd(nc_wq, in2, core_ids=core_ids)
    wq_full = np.empty((DIN, DOUT), dtype=fp8np)
    for c in core_ids:
        wq_full[c * WSEG:(c + 1) * WSEG, :] = r2.results[c]["wq8"]

    # Pair-interleaved layout for the fp8-pair DMA transpose convention:
    # wq_dr[p, s, b, o] = wq_full[s*256 + 2p + b, o].
    wq_dr = np.ascontiguousarray(
        wq_full.reshape(16, 128, 2, DOUT).transpose(1, 0, 2, 3))
    in3 = []
    for c in core_ids:
        tg = c // OG
        xsh = x[tg * TSH:(tg + 1) * TSH, :]
        xsh = np.ascontiguousarray(
            xsh.reshape(NT, 128, DIN)[:, ::-1, :].reshape(TSH, DIN))
        in3.append({
            "xs": xsh,
            "wts8": wq_dr,
            "sca": sca,
        })
    r3 = run_bass_kernel_spmd(nc_main, in3, core_ids=core_ids)

    outp = np.empty((TOK, DOUT), dtype=np.float32)
    for c in core_ids:
        tg, og = c // OG, c % OG
        outp[tg * TSH:(tg + 1) * TSH, og * OSH:(og + 1) * OSH] = \
            r3.results[c]["out"].astype(np.float32)
    if _collect is not None:
        _collect["r1"] = r1
        _collect["r2"] = r2
        _collect["r3"] = r3
    return outp


# revision 21
# speedup vs baseline: 1.0063x; 1.0063x over previous
"""BitLinear (BitNet a4.8-style) Trainium2 kernel.

Computes  out = act_quant_int4(x) @ ste_ternary(w).T  for
x:[8192,4096] f32, w:[4096,4096] f32, on 8 NeuronCores.

Math structure exploited:
  - act_quant_int4(x) rows are  k/s_t  with integer k in [-7,7],
    s_t = 7/amax_t  (per-token).  The clip to [-8,7] is a no-op since
    |x*s| <= 7 by construction.
  - ste_ternary(w) = q * scale with q in {-1,0,1},
    scale = max(mean|w|, 1e-8)  (global scalar).
  - So out[t,o] = (scale * amax_t / 7) * sum_i k[t,i] * q[o,i].
    The inner sum is an exact small-integer dot product: we run it on the
    PE array in fp8 (e4m3 holds -8..8 and -1..1 exactly; DoubleRow fp8
    accumulates exactly in fp32 PSUM), then scale rows by
    f_t = scale*amax_t/7 during PSUM eviction (bf16 out, cast to f32 on
    host; 2^-9 relative rounding vs the 2e-2 gate).

Three launches on 8 cores:
  1. scale pass: per-core partial |w| sums over a 1/8 row shard of wT,
     reduced in 128-element chunks; host finishes the reduction in f64
     and forms the exact global ternary scale.
  2. w-quant pass: each core ternarizes a 1/8 row shard of wT into fp8
     {-1,0,+1}, passes split across DVE/GpSimd with the cast+store on
     the ACT engine/queue; host gathers the full quantized wT (16.7 MB).
  3. main pass, data-parallel over tokens x8: each core takes
     x[1024,4096] f32 + the full pre-quantized wT[4096,4096] fp8, int4-
     quantizes x on the fly (abs-max reduce -> scale -> round via the
     +-1.5*2^23 magic trick), xbar-transposes the fp8 activations in
     pairs (ACT queue), and runs DoubleRow fp8 matmuls, scaling rows
     into bf16 during PSUM eviction.  The sweep is oc-major so each
     PSUM bank is reused ~1.7us after its eviction starts; x loads and
     wq streaming share the SP queue (x0, x1, then wq so the PE can
     start early); output stores go through the GpSimd SWDGE queue so
     their semaphore waits never head-block a load.

w is transposed on the host once (input marshalling) so the contraction
dim lands on SBUF partitions for both operands.
"""

import numpy as np
from contextlib import ExitStack

import concourse.bacc as bacc
import concourse.bass as bass
import concourse.mybir as mybir
import concourse.tile as tile
from concourse.bass_utils import run_bass_kernel_spmd

F32 = mybir.dt.float32
BF16 = mybir.dt.bfloat16
FP8 = mybir.dt.float8e4
ALU = mybir.AluOpType
ACTF = mybir.ActivationFunctionType

TOK, DIN, DOUT = 8192, 4096, 4096
NCORES = 8
TG, OG = 8, 1            # token shards x out-feature shards (data parallel)
TSH = TOK // TG          # 1024 tokens per core
OSH = DOUT // OG         # 4096 out features per core
NT = TSH // 128          # 8 token tiles per core
NKQ = 8                  # w held in 8 chunks of 2 s-planes (pipelining)
WSEG = DIN // NCORES     # 512 wT rows per core in launches 1/2
MAGIC = 12582912.0       # 1.5*2^23: float add/sub round-to-nearest-int trick
CLAMP = float(np.nextafter(np.float32(1.5), np.float32(0.0)))
EPS = 1e-8

_CACHE = {}


def _build_scale_nc():
    """Launch 1: per-core partial |w| sums, in 128-element chunks so the
    fp32 accumulation error stays ~1e-7 relative (host finishes in f64)."""
    nc = bacc.Bacc("TRN2", target_bir_lowering=False, debug=False,
                   num_devices=NCORES)
    wseg = nc.dram_tensor("wseg", [WSEG, DIN], F32,
                          kind="ExternalInput").ap()
    psums = nc.dram_tensor("psums", [128, 128], F32,
                           kind="ExternalOutput").ap()
    with tile.TileContext(nc) as tc, ExitStack() as ctx:
        pool = ctx.enter_context(tc.tile_pool(name="w", bufs=4))
        spool = ctx.enter_context(tc.tile_pool(name="s", bufs=1))
        sums = spool.tile([128, 8, 16], F32)
        for i in range(8):
            r0, c0 = (i // 2) * 128, (i % 2) * 2048
            wt = pool.tile([128, 16, 128], F32)
            nc.sync.dma_start(
                out=wt,
                in_=wseg[r0:r0 + 128, c0:c0 + 2048].rearrange(
                    "p (a b) -> p a b", a=16))
            nc.vector.tensor_reduce(
                out=sums[:, i, :], in_=wt, axis=mybir.AxisListType.X,
                op=ALU.add, apply_absolute_value=True)
        nc.sync.dma_start(out=psums, in_=sums.rearrange("p a b -> p (a b)"))
    nc.compile()
    return nc


def _build_wquant_nc():
    """Launch 2: ternarize a [512, 4096] row shard of wT into fp8.
    round(clip(y,-1,1)) == round(clamp(y, +-CLAMP)) for |y|<=2.1."""
    nc = bacc.Bacc("TRN2", target_bir_lowering=False, debug=False,
                   num_devices=NCORES)
    wseg = nc.dram_tensor("wseg", [WSEG, DIN], F32,
                          kind="ExternalInput").ap()
    sca = nc.dram_tensor("sca", [128, 2], F32, kind="ExternalInput").ap()
    wq8 = nc.dram_tensor("wq8", [WSEG, DIN], FP8,
                         kind="ExternalOutput").ap()
    with tile.TileContext(nc) as tc, ExitStack() as ctx:
        const = ctx.enter_context(tc.tile_pool(name="const", bufs=1))
        pool = ctx.enter_context(tc.tile_pool(name="w", bufs=8))
        qpool = ctx.enter_context(tc.tile_pool(name="q", bufs=4))
        scat = const.tile([128, 2], F32)
        nc.sync.dma_start(out=scat, in_=sca)
        # 8 chunks; both tensor_scalar passes stay on DVE (it runs them
        # in 2x mode, ~1.1us per pass -- 16 passes fit under the 23.3us
        # load stream and same-engine chaining avoids cross-engine
        # semaphore latency); cast + store ride the ACT engine/queue.
        for i in range(8):
            r0, c0 = (i // 2) * 128, (i % 2) * 2048
            wt = pool.tile([128, 2048], F32, tag="wt")
            nc.sync.dma_start(
                out=wt, in_=wseg[r0:r0 + 128, c0:c0 + 2048])
            nc.vector.tensor_scalar(
                out=wt, in0=wt, scalar1=scat[:, 0:1], scalar2=CLAMP,
                op0=ALU.mult, op1=ALU.min)
            nc.vector.tensor_scalar(
                out=wt, in0=wt, scalar1=-CLAMP, scalar2=MAGIC,
                op0=ALU.max, op1=ALU.add)
            qt = qpool.tile([128, 2048], FP8, tag="qt")
            nc.scalar.activation(out=qt, in_=wt, func=ACTF.Copy,
                                 bias=-MAGIC, scale=1.0)
            nc.scalar.dma_start(out=wq8[r0:r0 + 128, c0:c0 + 2048], in_=qt)
    nc.compile()
    return nc


def _build_main_nc():
    nc = bacc.Bacc("TRN2", target_bir_lowering=False, debug=False,
                   num_devices=NCORES)
    xs = nc.dram_tensor("xs", [TSH, DIN], F32, kind="ExternalInput").ap()
    # Pre-quantized w in pair-interleaved layout: wts8[p, s, b, o] is
    # q_{o,i} for i = s*256 + 2p + b.  This matches what the fp8-pair
    # (uint16) xbar DMA transpose produces for the activations, so the
    # contraction index mapping agrees between lhsT and rhs.
    wts8 = nc.dram_tensor("wts8", [128, 16, 2, OSH], FP8,
                          kind="ExternalInput").ap()
    sca = nc.dram_tensor("sca", [128, 2], F32, kind="ExternalInput").ap()
    out = nc.dram_tensor("out", [TSH, OSH], BF16, kind="ExternalOutput").ap()

    with tile.TileContext(nc) as tc, ExitStack() as ctx:
        const = ctx.enter_context(tc.tile_pool(name="const", bufs=1))
        wqpool = ctx.enter_context(tc.tile_pool(name="wqp", bufs=NKQ))
        xpool = ctx.enter_context(tc.tile_pool(name="xp", bufs=3))
        k8pool = ctx.enter_context(tc.tile_pool(name="k8p", bufs=2))
        ktpool = ctx.enter_context(tc.tile_pool(name="ktp", bufs=6))
        smalls = ctx.enter_context(tc.tile_pool(name="smalls", bufs=4))
        opool = ctx.enter_context(tc.tile_pool(name="osb", bufs=2))
        psum_m = ctx.enter_context(
            tc.tile_pool(name="psm", bufs=8, space="PSUM"))

        scat = const.tile([128, 2], F32)
        nc.sync.dma_start(out=scat, in_=sca)
        w_scale = scat[:, 1:2]

        # Anti-diagonal permutation for reversing per-partition vectors
        # (SwInterleave reverses stationary columns; the host feeds token
        # rows pre-reversed so PSUM comes out ascending, and f crosses the
        # reversal via a tiny R @ f matmul).
        rmat = const.tile([128, 128], F32)
        nc.gpsimd.memset(rmat, 0.0)
        nc.gpsimd.affine_select(
            out=rmat, in_=rmat, compare_op=ALU.not_equal, fill=1.0,
            base=-127, pattern=[[1, 128]], channel_multiplier=1)

        xts = [None] * NT

        def load_x(t):
            xt = xpool.tile([128, DIN], F32, tag="xt", name=f"xt{t}")
            for h in range(2):
                nc.sync.dma_start(
                    out=xt[:, h * 2048:(h + 1) * 2048],
                    in_=xs[t * 128:(t + 1) * 128, h * 2048:(h + 1) * 2048])
            xts[t] = xt

        # SP queue order: x0, x1, x2, wq0..7, then one x per iteration.
        # x-tiles must land ~11us before their sweep starts (the
        # amax->quant->cast->transpose prep chain), and the wq chunks
        # gate tile 0's s-major sweep, which holds all 8 PSUM banks
        # until its last chunk arrives; x3..x7 stream in behind wq well
        # ahead of their sweeps.
        load_x(0)
        load_x(1)
        load_x(2)
        wq = []
        for q in range(NKQ):
            wqt = wqpool.tile([128, 2, 2, OSH], FP8, tag="wq",
                              name=f"wq{q}")
            nc.sync.dma_start(out=wqt, in_=wts8[:, 2 * q:2 * q + 2, :, :])
            wq.append(wqt)

        # Software pipeline: iteration t preps tile t (quant/transpose)
        # and runs the matmul sweep + evictions for tile t-1.
        kts_all = [None] * NT
        fap_all = [None] * NT

        def prep(tt):
            xt = xts[tt]
            amax2 = smalls.tile([128, 2], F32, tag="amax2")
            for h in range(2):
                nc.vector.tensor_reduce(
                    out=amax2[:, h:h + 1],
                    in_=xt[:, h * 2048:(h + 1) * 2048],
                    axis=mybir.AxisListType.X, op=ALU.max,
                    apply_absolute_value=True)
            amax = smalls.tile([128, 1], F32, tag="amax")
            nc.vector.tensor_reduce(
                out=amax, in_=amax2, axis=mybir.AxisListType.X, op=ALU.max)
            nc.vector.tensor_scalar_max(amax, amax, EPS)
            s_ap = smalls.tile([128, 1], F32, tag="s_ap")
            nc.vector.reciprocal(out=s_ap, in_=amax)        # 1/amax
            nc.vector.tensor_scalar_mul(s_ap, s_ap, 7.0)    # s = 7/amax
            f_ap = smalls.tile([128, 1], F32, tag="f_ap")
            nc.vector.tensor_scalar(
                out=f_ap, in0=amax, scalar1=1.0 / 7.0, scalar2=w_scale,
                op0=ALU.mult, op1=ALU.mult)                 # scale*amax/7
            fap_all[tt] = f_ap
            # y = x*s + MAGIC (in-place; integer part is k+MAGIC) on the
            # otherwise-idle GpSimd; ACT subtracts MAGIC and casts to fp8;
            # the ACT-queue xbar DMA then block-transposes fp8 PAIRS (as
            # uint16): kt[p, s, t] = (k[t, s*256+2p], k[t, s*256+2p+1]).
            k8 = k8pool.tile([128, DIN], FP8, tag="k8")
            kts = [ktpool.tile([128, 8, 128], BF16, tag="kt",
                               name=f"kt{tt}_{h}") for h in range(2)]
            for h in range(2):
                for ib in range(4):
                    c0 = h * 2048 + ib * 512
                    nc.vector.tensor_scalar(
                        out=xt[:, c0:c0 + 512], in0=xt[:, c0:c0 + 512],
                        scalar1=s_ap, scalar2=MAGIC,
                        op0=ALU.mult, op1=ALU.add)
                nc.scalar.activation(
                    out=k8[:, h * 2048:(h + 1) * 2048],
                    in_=xt[:, h * 2048:(h + 1) * 2048],
                    func=ACTF.Copy, bias=-MAGIC, scale=1.0)
                nc.scalar.dma_start(
                    out=kts[h],
                    in_=k8.bitcast(BF16)[:, h * 1024:(h + 1) * 1024],
                    transpose=True)
            kts_all[tt] = kts

        def sweep(tt):
            """DoubleRow fp8 matmuls for tile tt + per-oc evictions.

            The f-reversal matmul runs here (not in prep) so it never
            head-blocks the PE queue on a later tile's amax chain.
            Tile 0 runs s-major (each wq chunk consumed as it lands
            during the fill); later tiles run oc-major so each PSUM bank
            is freed ~12us before its next reuse.
            """
            kts = kts_all[tt]
            pss = [psum_m.tile([128, 512], F32, tag="psm",
                               name=f"ps{tt}_{i}")
                   for i in range(8)]
            f_rev = smalls.tile([128, 1], F32, tag="f_rev")

            def frev_mm():
                # f follows the (reversed) fed row order; PSUM rows come
                # out in token order, so reverse f with the permutation
                # matmul.  It borrows the first column of bank 7, which
                # is not accumulated into until the oc7 block ~11us
                # later, so it costs no PSUM slot and no PE stall.  The
                # copy-out rides GpSimd: like the evictions it is sweep-
                # side work, and GpSimd runs nothing x-gated, so it can
                # never head-block behind a late x tile.
                fp = pss[7][:, 0:1]
                nc.tensor.matmul(fp, rmat, fap_all[tt], start=True,
                                 stop=True)
                nc.vector.tensor_copy(out=f_rev, in_=fp)

            def mm(s, oc):
                lhsT = kts[s // 8][:, s % 8, :].bitcast(FP8).rearrange(
                    "p (i m) -> p i m", i=2)
                nc.tensor.matmul(
                    pss[oc], lhsT,
                    wq[s // 2][:, s % 2, :, oc * 512:(oc + 1) * 512],
                    start=(s == 0), stop=(s == 15),
                    perf_mode=mybir.MatmulPerfMode.DoubleRowSwInterleave)

            def evict(oc, osb):
                # GpSimd cannot read PSUM, so evictions split ACT/DVE.
                # They run in pass b, ~7us after the x-gated prep chain
                # on these engines has drained, so head-blocking is
                # bounded to catch-up transients.
                j = oc % 4
                if oc % 2 == 0:
                    nc.scalar.activation(
                        out=osb[:, j * 512:(j + 1) * 512], in_=pss[oc],
                        func=ACTF.Copy, bias=0.0, scale=f_rev)
                else:
                    nc.vector.tensor_scalar(
                        out=osb[:, j * 512:(j + 1) * 512], in0=pss[oc],
                        scalar1=f_rev, scalar2=None, op0=ALU.mult)

            def store(half, osb):
                nc.gpsimd.dma_start(
                    out=out[tt * 128:(tt + 1) * 128,
                            half * 2048:(half + 1) * 2048],
                    in_=osb)

            if tt == 0:
                frev_mm()
                for s in range(16):
                    for oc in range(8):
                        mm(s, oc)
                for half in range(2):
                    osb = opool.tile([128, 2048], BF16, tag="osb")
                    for oc in range(half * 4, half * 4 + 4):
                        evict(oc, osb)
                    store(half, osb)
            else:
                # Two half-K passes: pass a (s 0..7) needs only kt[h0],
                # giving the h1 quant/cast/transpose chain ~7us of slack;
                # pass b completes the accumulation and evicts per-oc.
                osb = None
                for oc in range(8):
                    for s in range(8):
                        mm(s, oc)
                    if oc == 0:
                        frev_mm()
                for oc in range(8):
                    if oc % 4 == 0:
                        osb = opool.tile([128, 2048], BF16, tag="osb")
                    for s in range(8, 16):
                        mm(s, oc)
                    evict(oc, osb)
                    if oc % 4 == 3:
                        store(oc // 4, osb)

        prep(0)
        prep(1)
        sweep(0)
        for tt in range(2, NT):
            if tt + 1 < NT:
                load_x(tt + 1)
            prep(tt)
            sweep(tt - 1)
        sweep(NT - 1)
    nc.compile()
    return nc


def _get_ncs():
    if "scale" not in _CACHE:
        _CACHE["scale"] = _build_scale_nc()
    if "wquant" not in _CACHE:
        _CACHE["wquant"] = _build_wquant_nc()
    if "main" not in _CACHE:
        _CACHE["main"] = _build_main_nc()
    return _CACHE["scale"], _CACHE["wquant"], _CACHE["main"]


def kernel(x: np.ndarray, latent_weight: np.ndarray,
           _collect=None) -> np.ndarray:
    x = np.ascontiguousarray(x, dtype=np.float32)
    wT = np.ascontiguousarray(latent_weight.T.astype(np.float32))
    nc_scale, nc_wq, nc_main = _get_ncs()
    core_ids = list(range(NCORES))
    fp8np = mybir.dt.np(FP8)

    segs = [np.ascontiguousarray(wT[c * WSEG:(c + 1) * WSEG, :])
            for c in core_ids]
    in1 = [{"wseg": segs[c]} for c in core_ids]
    r1 = run_bass_kernel_spmd(nc_scale, in1, core_ids=core_ids)
    total = np.float64(0.0)
    for c in core_ids:
        total += r1.results[c]["psums"].astype(np.float64).sum()
    mean = np.float32(total / (DIN * DOUT))
    scale = np.maximum(mean, np.float32(EPS))
    inv_scale = np.float32(1.0) / scale

    sca = np.empty((128, 2), dtype=np.float32)
    sca[:, 0] = inv_scale
    sca[:, 1] = scale
    in2 = [{"wseg": segs[c], "sca": sca} for c in core_ids]
    r2 = run_bass_kernel_spmd(nc_wq, in2, core_ids=core_ids)
    wq_full = np.empty((DIN, DOUT), dtype=fp8np)
    for c in core_ids:
        wq_full[c * WSEG:(c + 1) * WSEG, :] = r2.results[c]["wq8"]

    # Pair-interleaved layout for the fp8-pair DMA transpose convention:
    # wq_dr[p, s, b, o] = wq_full[s*256 + 2p + b, o].
    wq_dr = np.ascontiguousarray(
        wq_full.reshape(16, 128, 2, DOUT).transpose(1, 0, 2, 3))
    in3 = []
    for c in core_ids:
        tg = c // OG
        xsh = x[tg * TSH:(tg + 1) * TSH, :]
        xsh = np.ascontiguousarray(
            xsh.reshape(NT, 128, DIN)[:, ::-1, :].reshape(TSH, DIN))
        in3.append({
            "xs": xsh,
            "wts8": wq_dr,
            "sca": sca,
        })
    r3 = run_bass_kernel_spmd(nc_main, in3, core_ids=core_ids)

    outp = np.empty((TOK, DOUT), dtype=np.float32)
    for c in core_ids:
        tg, og = c // OG, c % OG
        outp[tg * TSH:(tg + 1) * TSH, og * OSH:(og + 1) * OSH] = \
            r3.results[c]["out"].astype(np.float32)
    if _collect is not None:
        _collect["r1"] = r1
        _collect["r2"] = r2
        _collect["r3"] = r3
    return outp
